# revision 18
# baseline (speedup 1.0000x reference)
"""GroupLinear (MoE routing) Trainium2 kernel.

Problem: x [8192, 1024] f32, indices [8192] int64 in [0,8),
W [8*2048, 1024] f32, b [8*2048] f32.
out[n] = x[n] @ W[g*2048:(g+1)*2048].T + b[g*2048:(g+1)*2048],  g = indices[n].

Strategy: expert-parallel across the 8 NeuronCores. Core g owns group g's
weight slice only (8MB instead of the full 64MB), and processes exactly the
rows routed to group g. Row routing (argsort of indices) happens on host;
the device kernel is a dense [C_pad, 1024] @ [1024, 2048] + bias matmul in
float32r (full PE rate, near-fp32 precision).

Host pre-layout puts both operands K-major *and* partition-major so every
DMA moves long contiguous lines per partition:
  x_r [128, 8*C_pad] : x_r[p, kc*C_pad + c] = x[rows[c], kc*128+p]
  w_r [128, 8*2048]  : w_r[p, kc*2048 + o]  = W_g[o, kc*128+p]
Loads go on the Sync HWDGE ring, stores + bias on the Scalar HWDGE ring so
store semaphore waits never block load issue. A junk-matmul warmup burst
lifts the PE HAM clock gate before the real matmuls arrive.
"""

import os
import sys

sys.path.insert(0, "/opt/trn_rl_repo")

import numpy as np

import concourse.bass as bass
import concourse.bacc as bacc
import concourse.mybir as mybir
import concourse.tile as tile
from concourse.bass_utils import run_bass_kernel_spmd
from concourse.tile_rust import add_dep_helper

N = 8192
IN_F = 1024
OUT_F = 2048
G = 8
NCORES = 8
P = 128
NB_SZ = 512  # matmul moving-dim / PSUM bank free size (fp32)
N_WARMUP = 10  # junk matmuls to lift the PE clock gate during load phase

LAST_EXEC_NS = None
LAST_RESULTS = None

_nc_cache = {}


def _build_nc(c_pad: int):
    """Build the per-core Bass program for C_pad routed rows."""
    assert c_pad % P == 0
    kc_n = IN_F // P       # 8 k-chunks
    nb_n = OUT_F // NB_SZ  # 4 output-feature blocks
    mb_n = c_pad // P      # row blocks

    nc = bacc.Bacc("TRN2", target_bir_lowering=False, debug=False)
    f32r = mybir.dt.float32r

    x_r = nc.dram_tensor("x_r", [P, c_pad * IN_F // P], f32r, kind="ExternalInput")
    w_r = nc.dram_tensor("w_r", [P, kc_n * OUT_F], f32r, kind="ExternalInput")
    bias = nc.dram_tensor("bias", [1, OUT_F], mybir.dt.float32, kind="ExternalInput")
    out = nc.dram_tensor("out", [c_pad, OUT_F], mybir.dt.float32, kind="ExternalOutput")

    with tile.TileContext(nc) as tc:
        with (
            tc.tile_pool(name="wp", bufs=1) as wp,
            tc.tile_pool(name="xp", bufs=1) as xp,
            tc.tile_pool(name="bp", bufs=1) as bp,
            tc.tile_pool(name="op", bufs=mb_n * nb_n) as op,
            tc.tile_pool(name="pp", bufs=7, space="PSUM") as pp,
            tc.tile_pool(name="warm", bufs=1) as warmp,
            tc.tile_pool(name="warmps", bufs=1, space="PSUM") as warmpp,
        ):
            # -- PE warmup: junk matmuls with no data deps run immediately,
            # flipping the HAM clock gate to 2.4GHz while loads stream in.
            warm_sb = warmp.tile([P, NB_SZ], mybir.dt.bfloat16, name="warm_sb",
                                 tag="warm_sb")
            nc.vector.memset(warm_sb[:], 0.0)
            warm_ps = warmpp.tile([P, NB_SZ], mybir.dt.float32, name="warm_ps",
                                  tag="warm_ps")
            # 8 long matmuls flip the clock gate (~3.4us), then short ones
            # keep PE busy (fine-grained, so real work queues <110ns) until
            # the first x/w pieces land.
            for i in range(8):
                nc.tensor.matmul(
                    warm_ps[:], warm_sb[:, 0:P], warm_sb[:],
                    start=(i == 0), stop=(i == 7),
                )
            for i in range(30):
                nc.tensor.matmul(
                    warm_ps[:, 0:P], warm_sb[:, 0:P], warm_sb[:, 0:P],
                    start=True, stop=True,
                )

            # All loads on the Sync HWDGE ring (one serial delivery stream,
            # full HBM bandwidth), emitted in consumption-deadline order.
            # Data travels the ring in order, so pieces are interleaved:
            # w_nb0 quarters with x_mb0 first, then x pieces paced against
            # the nb0 column, w_nb1 halves mid-column, then w_nb2/w_nb3.
            # bias rides the otherwise-idle Scalar ring (deadline ~24us).
            x_sb = [None] * mb_n
            w_sb = [None] * nb_n
            for nb in range(nb_n):
                w_sb[nb] = wp.tile([P, kc_n * NB_SZ], f32r, name=f"w{nb}",
                                   tag=f"w{nb}")
            for mb in range(mb_n):
                x_sb[mb] = xp.tile([P, IN_F], f32r, name=f"x{mb}", tag=f"x{mb}")

            def load_w(nb, lo, hi):  # [lo, hi) in units of NB_SZ columns
                base = nb * kc_n * NB_SZ
                return nc.sync.dma_start(
                    w_sb[nb][:, lo * NB_SZ:hi * NB_SZ],
                    w_r[:, base + lo * NB_SZ:base + hi * NB_SZ],
                )

            def load_x(mb):
                nc.sync.dma_start(
                    x_sb[mb][:], x_r[:, mb * IN_F:(mb + 1) * IN_F]
                )

            bias_sb = bp.tile([P, OUT_F], mybir.dt.float32, tag="bias")
            nc.scalar.dma_start(bias_sb[:], bias[0:1, :].to_broadcast((P, OUT_F)))

            xq = list(range(mb_n))  # x pieces not yet emitted

            def pop_x(k):
                for _ in range(min(k, len(xq))):
                    load_x(xq.pop(0))

            load_w(0, 0, 2)
            pop_x(1)
            load_w(0, 2, 4)
            load_w(0, 4, 6)
            pop_x(1)
            load_w(0, 6, 8)
            pop_x(3)
            load_w(1, 0, 4)
            pop_x(len(xq))
            load_w(1, 4, 8)
            load_w(2, 0, 8)
            w_last = load_w(3, 0, 8)

            for nb in range(nb_n):
                for mb in range(mb_n):
                    psum = pp.tile([P, NB_SZ], mybir.dt.float32,
                                   name=f"ps{nb}_{mb}", tag="psum")
                    for kc in range(kc_n):
                        nc.tensor.matmul(
                            psum[:],
                            x_sb[mb][:, kc * P:(kc + 1) * P],
                            w_sb[nb][:, kc * NB_SZ:(kc + 1) * NB_SZ],
                            start=(kc == 0),
                            stop=(kc == kc_n - 1),
                        )
                    ot = op.tile([P, NB_SZ], mybir.dt.float32,
                                 name=f"ot{nb}_{mb}", tag="ot")
                    nc.vector.tensor_add(
                        ot[:], psum[:], bias_sb[:, nb * NB_SZ:(nb + 1) * NB_SZ]
                    )
                    st = nc.scalar.dma_start(
                        out[mb * P:(mb + 1) * P, nb * NB_SZ:(nb + 1) * NB_SZ],
                        ot[:],
                    )
                    add_dep_helper(st.ins, w_last.ins,
                                   reason="defer stores behind W loads")

    nc.compile()
    return nc


def _get_nc(c_pad: int):
    nc = _nc_cache.get(c_pad)
    if nc is None:
        nc = _build_nc(c_pad)
        _nc_cache[c_pad] = nc
    return nc


def kernel(x, indices, W, b):
    global LAST_EXEC_NS, LAST_RESULTS

    x = np.ascontiguousarray(np.asarray(x, dtype=np.float32))
    W = np.ascontiguousarray(np.asarray(W, dtype=np.float32))
    b = np.asarray(b, dtype=np.float32)
    idx = np.asarray(indices).astype(np.int64)

    order = np.argsort(idx, kind="stable")
    counts = np.bincount(idx, minlength=G)
    offs = np.zeros(G + 1, dtype=np.int64)
    np.cumsum(counts, out=offs[1:])

    c_pad = max(P, int(-(-counts.max() // P)) * P)
    kc_n = IN_F // P
    nc = _get_nc(c_pad)

    rows = [order[offs[g]:offs[g + 1]] for g in range(G)]
    mb_n = c_pad // P
    nb_n = OUT_F // NB_SZ
    in_maps = []
    for g in range(G):
        # x_r [128, mb_n*1024]: piece mb holds x_r[p, mb*1024 + kc*128 + c]
        #   = x[rows[mb*128+c], kc*128+p]
        xT = np.zeros((IN_F, c_pad), dtype=np.float32)
        cg = int(counts[g])
        if cg:
            xT[:, :cg] = x[rows[g]].T
        xr = np.ascontiguousarray(
            xT.reshape(kc_n, P, mb_n, P)
            .transpose(1, 2, 0, 3)
            .reshape(P, mb_n * IN_F)
        )
        # w_r [128, nb_n*8*512]: piece nb holds w_r[p, nb*4096 + kc*512 + o]
        #   = W_g[nb*512+o, kc*128+p]
        wT = W[g * OUT_F:(g + 1) * OUT_F, :].T  # [1024, 2048]
        wr = np.ascontiguousarray(
            wT.reshape(kc_n, P, nb_n, NB_SZ)
            .transpose(1, 2, 0, 3)
            .reshape(P, kc_n * OUT_F)
        )
        bg = np.ascontiguousarray(b[g * OUT_F:(g + 1) * OUT_F]).reshape(1, OUT_F)
        in_maps.append({"x_r": xr, "w_r": wr, "bias": bg})

    trace = bool(int(os.environ.get("KERNEL_TRACE", "0")))
    res = run_bass_kernel_spmd(nc, in_maps, list(range(NCORES)), trace=trace)
    LAST_EXEC_NS = res.exec_time_ns
    LAST_RESULTS = res

    out = np.empty((N, OUT_F), dtype=np.float32)
    for g in range(G):
        cg = int(counts[g])
        if cg:
            out[rows[g]] = res.results[g]["out"][:cg]
    return out
